# revision 4
# baseline (speedup 1.0000x reference)
"""Trainium2 Bass kernel for nn_LogBessel: out = log(I_31(kappa) + 1e-10).

Math: instead of the reference's 128-term log-space power series, use the
exact identity (uniform asymptotic / Debye structure)

    ln I_nu(x) = W - nu*ln(nu + W) + nu*ln(x) + P(y),
    W = sqrt(nu^2 + x^2),  y = ln(W^2),  nu = 31

where P(y) = -0.25*y - 0.5*ln(2*pi) + ln(sum_k u_k/nu^k) is smooth and tiny;
it is fitted offline as a degree-4 polynomial in y over y in [ln 961,
ln 3461] (max fit error 6.7e-7, fp32 Horner error 1.4e-6 -- both far below
the fp32 noise of the reference itself).

Engine split per [128 x 2048] chunk:
  ScalarE (ACT, one natural_log_exp table set, no table switching):
      L = Ln(x); y = Ln(x^2+961); W = Exp(0.5*y); q = Ln(W+31);
      iv = Exp(g); out = Ln(iv + 1e-10)
  GpSimd:  x^2 = x*x (tensor_tensor), final g-sum (scalar_tensor_tensor)
  VectorE: Horner for P(y) + assembly, via fused scalar_tensor_tensor
           (out = (in0 op0 scalar) op1 in1).

The final Exp/Ln pair reproduces the reference's exp(log_iv) + eps -> log
structure, so the small-x regime (output == log(1e-10)) matches exactly.

Sharding: trivially data-parallel; 4096 rows split into 8 blocks of 512,
one per NeuronCore (same SPMD program, different data).
"""

import numpy as np

from concourse import bacc, mybir, tile
from concourse import bass_utils

F32 = mybir.dt.float32
AF = mybir.ActivationFunctionType
OP = mybir.AluOpType

N_CORES = 8
ROWS, COLS = 4096, 4096
SH_ROWS = ROWS // N_CORES          # 512 rows per core
P = 128                            # SBUF partitions
FD = 2048                          # free-dim chunk size
ROW_BLOCKS = SH_ROWS // P          # 4
COL_BLOCKS = COLS // FD            # 2

# deg-4 fit of P(y) on [ln 961, ln 3461], power basis (see docstring)
A0 = -3.087667582403775
A1 = 0.7840119052482061
A2 = -0.18577208264273426
A3 = 0.014913698452924522
A4 = -0.00045134658423458393
EPS = 1e-10

_nc_cache = None


def _build():
    nc = bacc.Bacc("TRN2", target_bir_lowering=False, debug=False)
    x = nc.dram_tensor("x", [SH_ROWS, COLS], F32, kind="ExternalInput").ap()
    y = nc.dram_tensor("y", [SH_ROWS, COLS], F32, kind="ExternalOutput").ap()

    # activation() requires float biases to exist as [128,1] const SBUF
    # tensors; register ours the same way Bass.__init__ registers 0.0/1.0.
    for val in (961.0, 31.0, EPS, A0):
        t = nc.alloc_sbuf_tensor(f"const-f32-{val}", [128, 1], F32)
        nc.gpsimd.memset(t.ap(), val)
        nc.const_aps.aps[(F32, val)] = t.ap()
    nc.all_engine_barrier()

    with tile.TileContext(nc) as tc:
        with tc.tile_pool(name="p", bufs=2) as pool:
            for c in range(ROW_BLOCKS):
                for d in range(COL_BLOCKS):
                    rs = slice(c * P, (c + 1) * P)
                    cs = slice(d * FD, (d + 1) * FD)

                    tx = pool.tile([P, FD], F32, tag="x")
                    nc.sync.dma_start(tx[:], x[rs, cs])

                    tx2 = pool.tile([P, FD], F32, tag="x2")
                    nc.gpsimd.tensor_tensor(tx2[:], tx[:], tx[:], OP.mult)
                    tL = pool.tile([P, FD], F32, tag="L")
                    nc.scalar.activation(tL[:], tx[:], AF.Ln)
                    ty = pool.tile([P, FD], F32, tag="y")
                    nc.scalar.activation(ty[:], tx2[:], AF.Ln, bias=961.0)
                    tW = pool.tile([P, FD], F32, tag="W")
                    nc.scalar.activation(tW[:], ty[:], AF.Exp, scale=0.5)
                    tq = pool.tile([P, FD], F32, tag="q")
                    nc.scalar.activation(tq[:], tW[:], AF.Ln, bias=31.0)

                    # Horner for P(y): H = a4*y; H = (H + a_j)*y
                    tG = pool.tile([P, FD], F32, tag="G")
                    nc.vector.tensor_scalar_mul(tG[:], ty[:], A4)
                    nc.vector.scalar_tensor_tensor(
                        tG[:], tG[:], A3, ty[:], op0=OP.add, op1=OP.mult)
                    nc.vector.scalar_tensor_tensor(
                        tG[:], tG[:], A2, ty[:], op0=OP.add, op1=OP.mult)
                    nc.vector.scalar_tensor_tensor(
                        tG[:], tG[:], A1, ty[:], op0=OP.add, op1=OP.mult)

                    # assembly: g = W - 31*ln(31+W) + 31*ln(x) + H + a0
                    ts_ = pool.tile([P, FD], F32, tag="s")
                    nc.vector.scalar_tensor_tensor(
                        ts_[:], tq[:], -31.0, tW[:], op0=OP.mult, op1=OP.add)
                    nc.vector.scalar_tensor_tensor(
                        ts_[:], tL[:], 31.0, ts_[:], op0=OP.mult, op1=OP.add)
                    tg = pool.tile([P, FD], F32, tag="g")
                    nc.gpsimd.tensor_tensor(tg[:], tG[:], ts_[:], OP.add)

                    # out = ln(exp(g + a0) + eps)  (a0 folded into Exp bias;
                    # same exp -> +eps -> log structure as the reference)
                    to = pool.tile([P, FD], F32, tag="o")
                    nc.scalar.activation(to[:], tg[:], AF.Exp, bias=A0)
                    nc.scalar.activation(to[:], to[:], AF.Ln, bias=EPS)

                    nc.sync.dma_start(y[rs, cs], to[:])

    nc.compile()
    return nc


def _get_nc():
    global _nc_cache
    if _nc_cache is None:
        _nc_cache = _build()
    return _nc_cache


def kernel(kappa: np.ndarray) -> np.ndarray:
    kappa = np.ascontiguousarray(np.asarray(kappa, dtype=np.float32))
    assert kappa.shape == (ROWS, COLS)
    nc = _get_nc()
    in_maps = [
        {"x": kappa[i * SH_ROWS:(i + 1) * SH_ROWS]} for i in range(N_CORES)
    ]
    res = bass_utils.run_bass_kernel_spmd(
        nc, in_maps, core_ids=list(range(N_CORES)))
    out = np.concatenate([res.results[i]["y"] for i in range(N_CORES)], axis=0)
    return out.astype(np.float32)


# revision 6
# speedup vs baseline: 1.2715x; 1.2715x over previous
"""Trainium2 Bass kernel for nn_LogBessel: out = log(I_31(kappa) + 1e-10).

Math: instead of the reference's 128-term log-space power series, use the
exact identity (uniform asymptotic / Debye structure)

    ln I_nu(x) = W - nu*ln(nu + W) + nu*ln(x) + P(y),
    W = sqrt(nu^2 + x^2),  y = ln(W^2),  nu = 31

where P(y) = -0.25*y - 0.5*ln(2*pi) + ln(sum_k u_k/nu^k) is smooth and tiny;
it is fitted offline as a degree-4 polynomial in y over y in [ln 961,
ln 3461] (max fit error 6.7e-7, fp32 Horner error 1.4e-6 -- both far below
the fp32 noise of the reference itself).

Engine split per [128 x 2048] chunk:
  ScalarE (ACT, one natural_log_exp table set, no table switching):
      L = Ln(x); y = Ln(x^2+961); W = Exp(0.5*y); q = Ln(W+31);
      iv = Exp(g); out = Ln(iv + 1e-10)
  (GpSimd stays idle: it shares SBUF ports with VectorE, so offloading
   elementwise work there slows VectorE down.)
  VectorE: Horner for P(y) + assembly, via fused scalar_tensor_tensor
           (out = (in0 op0 scalar) op1 in1).

The final Exp/Ln pair reproduces the reference's exp(log_iv) + eps -> log
structure, so the small-x regime (output == log(1e-10)) matches exactly.

Sharding: trivially data-parallel; 4096 rows split into 8 blocks of 512,
one per NeuronCore (same SPMD program, different data).
"""

import numpy as np

from concourse import bacc, mybir, tile
from concourse import bass_utils

F32 = mybir.dt.float32
AF = mybir.ActivationFunctionType
OP = mybir.AluOpType

N_CORES = 8
ROWS, COLS = 4096, 4096
SH_ROWS = ROWS // N_CORES          # 512 rows per core
P = 128                            # SBUF partitions
FD = 2048                          # free-dim chunk size
ROW_BLOCKS = SH_ROWS // P          # 4
COL_BLOCKS = COLS // FD            # 2

# deg-4 fit of P(y) on [ln 961, ln 3461], power basis (see docstring)
A0 = -3.087667582403775
A1 = 0.7840119052482061
A2 = -0.18577208264273426
A3 = 0.014913698452924522
A4 = -0.00045134658423458393
EPS = 1e-10

_nc_cache = None



_ACT_SET = "natural_log_exp_and_others"


def _force_single_act_set():
    """Make ln/exp/square resolvable only from natural_log_exp_and_others so
    walrus's per-function set assignment cannot ping-pong table loads."""
    import json, tempfile, os
    try:
        from neuronxcc.driver.jobs.support import FindActInfo
        from neuronxcc.driver.jobs import WalrusDriver as WD
    except ImportError:
        return
    if getattr(FindActInfo, "_logbessel_patched", False):
        return
    orig = FindActInfo.findActInfoFile

    def patched(package_dir, arch):
        path = orig(package_dir, arch)
        try:
            import shutil
            # table .bin blobs are resolved relative to the json, so clone
            # the whole pwp_bin dir and patch the json inside the clone
            dst = os.path.join(tempfile.gettempdir(), "pwp_single_set")
            if not os.path.isdir(dst):
                shutil.copytree(os.path.dirname(path), dst)
            d = json.load(open(path))
            for s in d.get("act_func_sets", []):
                if s.get("name") != _ACT_SET:
                    for fn in ("ln", "exp", "square"):
                        s.get("act", {}).pop(fn, None)
            out = os.path.join(dst, "act_info.json")
            with open(out, "w") as f:
                json.dump(d, f)
            return out
        except Exception:
            return path

    patched._logbessel_patched = True
    FindActInfo._logbessel_patched = True
    FindActInfo.findActInfoFile = patched
    WD.findActInfoFile = patched


def _build():
    _force_single_act_set()
    nc = bacc.Bacc("TRN2", target_bir_lowering=False, debug=False)
    x = nc.dram_tensor("x", [SH_ROWS, COLS], F32, kind="ExternalInput").ap()
    y = nc.dram_tensor("y", [SH_ROWS, COLS], F32, kind="ExternalOutput").ap()

    # activation() requires float biases to exist as [128,1] const SBUF
    # tensors; register ours the same way Bass.__init__ registers 0.0/1.0.
    for val in (961.0, 31.0, EPS, A0):
        t = nc.alloc_sbuf_tensor(f"const-f32-{val}", [128, 1], F32)
        nc.gpsimd.memset(t.ap(), val)
        nc.const_aps.aps[(F32, val)] = t.ap()
    nc.all_engine_barrier()

    with tile.TileContext(nc) as tc:
        with tc.tile_pool(name="p", bufs=2) as pool:
            for c in range(ROW_BLOCKS):
                for d in range(COL_BLOCKS):
                    rs = slice(c * P, (c + 1) * P)
                    cs = slice(d * FD, (d + 1) * FD)

                    tx = pool.tile([P, FD], F32, tag="x")
                    nc.sync.dma_start(tx[:], x[rs, cs])

                    tx2 = pool.tile([P, FD], F32, tag="x2")
                    nc.scalar.activation(tx2[:], tx[:], AF.Square)
                    tL = pool.tile([P, FD], F32, tag="L")
                    nc.scalar.activation(tL[:], tx[:], AF.Ln)
                    ty = pool.tile([P, FD], F32, tag="y")
                    nc.scalar.activation(ty[:], tx2[:], AF.Ln, bias=961.0)
                    tW = pool.tile([P, FD], F32, tag="W")
                    nc.scalar.activation(tW[:], ty[:], AF.Exp, scale=0.5)
                    tq = pool.tile([P, FD], F32, tag="q")
                    nc.scalar.activation(tq[:], tW[:], AF.Ln, bias=31.0)

                    # Horner for P(y): H = a4*y; H = (H + a_j)*y
                    tG = pool.tile([P, FD], F32, tag="G")
                    nc.vector.tensor_scalar_mul(tG[:], ty[:], A4)
                    nc.vector.scalar_tensor_tensor(
                        tG[:], tG[:], A3, ty[:], op0=OP.add, op1=OP.mult)
                    nc.vector.scalar_tensor_tensor(
                        tG[:], tG[:], A2, ty[:], op0=OP.add, op1=OP.mult)
                    nc.vector.scalar_tensor_tensor(
                        tG[:], tG[:], A1, ty[:], op0=OP.add, op1=OP.mult)

                    # assembly: g = W - 31*ln(31+W) + 31*ln(x) + H + a0
                    ts_ = pool.tile([P, FD], F32, tag="s")
                    nc.vector.scalar_tensor_tensor(
                        ts_[:], tq[:], -31.0, tW[:], op0=OP.mult, op1=OP.add)
                    nc.vector.scalar_tensor_tensor(
                        ts_[:], tL[:], 31.0, ts_[:], op0=OP.mult, op1=OP.add)
                    tg = pool.tile([P, FD], F32, tag="g")
                    nc.vector.tensor_tensor(tg[:], tG[:], ts_[:], OP.add)

                    # out = ln(exp(g + a0) + eps)  (a0 folded into Exp bias;
                    # same exp -> +eps -> log structure as the reference)
                    to = pool.tile([P, FD], F32, tag="o")
                    nc.scalar.activation(to[:], tg[:], AF.Exp, bias=A0)
                    nc.scalar.activation(to[:], to[:], AF.Ln, bias=EPS)

                    nc.sync.dma_start(y[rs, cs], to[:])

    nc.compile()
    return nc


def _get_nc():
    global _nc_cache
    if _nc_cache is None:
        _nc_cache = _build()
    return _nc_cache


def kernel(kappa: np.ndarray) -> np.ndarray:
    kappa = np.ascontiguousarray(np.asarray(kappa, dtype=np.float32))
    assert kappa.shape == (ROWS, COLS)
    nc = _get_nc()
    in_maps = [
        {"x": kappa[i * SH_ROWS:(i + 1) * SH_ROWS]} for i in range(N_CORES)
    ]
    res = bass_utils.run_bass_kernel_spmd(
        nc, in_maps, core_ids=list(range(N_CORES)))
    out = np.concatenate([res.results[i]["y"] for i in range(N_CORES)], axis=0)
    return out.astype(np.float32)
